# revision 43
# baseline (speedup 1.0000x reference)
"""Distributed causal multi-head attention for TRN2, 8 NeuronCores.

Sharding: core c (0..7) handles batch c//4 and heads 4*(c%4)..4*(c%4)+3
(tensor-parallel over heads x data-parallel over batch).

Per-core pipeline (all matmuls bf16, fp32 PSUM accumulate):
  1. QKV projections from host-pretransposed xT:
       QT/KT[k,s] = (W.T x.T) with W tiles stationary;
       V[s,k] with xT tiles stationary (4 heads packed in the free dim).
  2. Attention per head, scores transposed: ST[s,q] = KT.T @ QT.
     exp on ACT, causal masking by precomputed 0/1 tiles, then
     z[q,k] and the softmax row-sum r[q] in ONE matmul per
     (q-tile, s-tile): rhs = [V | ones] (129 columns).
     Normalize z by 1/r (per-partition scalar), PE-transpose to zT and
     DMA to the AllGather buffer in [hk_local, q] layout.
  3. AllGather over the 4-core batch group: every core gets the full
     zT [16*128, 2048] for its batch.
  4. Output projection, d-sharded: each core's wo input holds only its
     512 W_O columns, so out[all q, d_slice] = z_flat @ W_O[:, slice].
     The graph is identical on all cores; per-core behavior comes only
     from input data (SPMD-safe).

Host: shards/casts/transposes inputs, adds bias corrections
(b_O + sum_h b_V[h] @ W_O[h] is a constant row because softmax rows sum
to 1; b_Q/b_K are folded into the QT/KT PSUM evacuation on device).
"""
import math
import os

import numpy as np
import ml_dtypes

import concourse.bacc as bacc
import concourse.mybir as mybir
from concourse import tile, masks
from concourse.bass_utils import run_bass_kernel_spmd

BF16 = mybir.dt.bfloat16
F32 = mybir.dt.float32
NPBF16 = ml_dtypes.bfloat16

B = 2
SEQ = 2048
D_MODEL = 2048
N_HEADS = 16
D_HEAD = 128
HPC = 4              # heads per core
NCORES = 8
GROUPS = [[0, 1, 2, 3], [4, 5, 6, 7]]
NDT = D_MODEL // 128   # 16 d-model tiles
NST = SEQ // 128       # 16 seq tiles
NQC = SEQ // 512       # 4 q-chunks
QSL = SEQ // 4         # 512 per-core q-slice for output projection
SCALE = 1.0 / math.sqrt(D_HEAD)

LAST_EXEC_NS = None


def build_nc():
    nc = bacc.Bacc(None, num_devices=NCORES, debug=False)

    xt_e = nc.declare_dram_parameter("xt", [D_MODEL, SEQ], BF16, isOutput=False)
    wq_e = nc.declare_dram_parameter("wq", [HPC * D_MODEL, D_HEAD], BF16, isOutput=False)
    wk_e = nc.declare_dram_parameter("wk", [HPC * D_MODEL, D_HEAD], BF16, isOutput=False)
    wv_e = nc.declare_dram_parameter("wv", [D_MODEL, HPC * D_HEAD], BF16, isOutput=False)
    wo_e = nc.declare_dram_parameter("wo", [N_HEADS * D_HEAD, QSL], BF16, isOutput=False)
    bq_e = nc.declare_dram_parameter("bq", [D_HEAD, HPC], F32, isOutput=False)
    bk_e = nc.declare_dram_parameter("bk", [D_HEAD, HPC], F32, isOutput=False)
    mk_e = nc.declare_dram_parameter("mk", [128, 4 * 512], BF16, isOutput=False)
    out_e = nc.declare_dram_parameter("out", [SEQ, QSL], F32, isOutput=True)

    # AllGather buffers: one per (local head, q-chunk). Quarters are
    # cheap on the CC engine and keep its queue from head-blocking the
    # final gathers that gate the output projection.
    agin = [[nc.dram_tensor(f"agin{h}_{j}", [D_HEAD, 512], BF16)
             for j in range(NQC)] for h in range(HPC)]
    agout = [[nc.dram_tensor(f"agout{h}_{j}", [4 * D_HEAD, 512], BF16)
              for j in range(NQC)] for h in range(HPC)]
    # the last (h3, j3) gather is split into 4 x 128-column pieces so the
    # final output-projection chains pipeline with the collective
    agin_p = [nc.dram_tensor(f"aginp{p}", [D_HEAD, 128], BF16)
              for p in range(4)]
    agout_p = [nc.dram_tensor(f"agoutp{p}", [4 * D_HEAD, 128], BF16)
               for p in range(4)]

    with tile.TileContext(nc) as tc:
        with tc.tile_pool(name="persist", bufs=1) as pp, \
             tc.tile_pool(name="xtp", bufs=NDT) as xt_pool, \
             tc.tile_pool(name="qkp", bufs=2) as qk_pool, \
             tc.tile_pool(name="vp", bufs=HPC) as v_pool, \
             tc.tile_pool(name="wvp", bufs=1) as wv_pool, \
             tc.tile_pool(name="pt", bufs=5) as pt_pool, \
             tc.tile_pool(name="zz", bufs=3) as z_pool, \
             tc.tile_pool(name="wo", bufs=1) as wo_pool, \
             tc.tile_pool(name="zg", bufs=2) as zg_pool, \
             tc.tile_pool(name="os", bufs=1) as out_pool, \
             tc.tile_pool(name="ps1", bufs=2, space="PSUM") as ps1, \
             tc.tile_pool(name="ps_st", bufs=2, space="PSUM") as ps_st, \
             tc.tile_pool(name="ps_zt", bufs=2, space="PSUM") as ps_zt, \
             tc.tile_pool(name="ps_r", bufs=2, space="PSUM") as ps_r:
            ones_blk = pp.tile([128, 128], BF16, tag="ones_blk")
            nc.vector.memset(ones_blk[:], 1.0)
            bq_sb = pp.tile([128, HPC], F32, tag="bq")
            nc.sync.dma_start(bq_sb[:], bq_e[:, :])
            bk_sb = pp.tile([128, HPC], F32, tag="bk")
            nc.sync.dma_start(bk_sb[:], bk_e[:, :])
            mk_sb = pp.tile([128, 4 * 512], BF16, tag="mk")
            nc.sync.dma_start(mk_sb[:], mk_e[:, :])

            xt_sb = []
            for dt in range(NDT):
                t = xt_pool.tile([128, SEQ], BF16, tag="xt", name=f"xt{dt}")
                eng = nc.sync if dt % 2 == 0 else nc.gpsimd
                eng.dma_start(t[:], xt_e[dt * 128:(dt + 1) * 128, :])
                xt_sb.append(t)

            # head 0's projection weights go first so its QT/KT chains can
            # start as soon as xt lands.
            wqk0 = []
            for nm, e in (("wq0", wq_e), ("wk0", wk_e)):
                t = qk_pool.tile([128, NDT, D_HEAD], BF16,
                                 tag="wq" if nm == "wq0" else "wk", name=nm)
                nc.gpsimd.dma_start(
                    t[:], e[0:D_MODEL, :].rearrange("(t p) k -> p t k", p=128))
                wqk0.append(t)

            # ---- V for all 4 heads (packed rhs) --------------------------
            v_sb = [v_pool.tile([128, NST, D_HEAD], BF16, tag="v",
                                name=f"v{h}") for h in range(HPC)]
            wv_sb = wv_pool.tile([128, NDT, HPC * D_HEAD], BF16, tag="wv")
            for g4 in range(4):
                nc.gpsimd.dma_start(
                    wv_sb[:, g4 * 4:(g4 + 1) * 4, :],
                    wv_e[g4 * 512:(g4 + 1) * 512, :]
                    .rearrange("(t p) k -> p t k", p=128))
            wo_sb = wo_pool.tile([128, N_HEADS, QSL], BF16, tag="wo")

            def emit_v_chains(st0, st1):
                for st in range(st0, st1):
                    psum = ps1.tile([128, 512], F32, tag="ps1",
                                    name=f"pv{st}")
                    for dt in range(NDT):
                        nc.tensor.matmul(
                            psum[:],
                            xt_sb[dt][:, st * 128:(st + 1) * 128],
                            wv_sb[:, dt, :],
                            start=(dt == 0), stop=(dt == NDT - 1))
                    for h in range(HPC):
                        nc.scalar.copy(
                            v_sb[h][:, st, 0:D_HEAD],
                            psum[:, h * 128:(h + 1) * 128])

            # ---- helpers -------------------------------------------------
            def finalize_chunk(st):
                """Chunk epilogue: broadcast the softmax denominator with a
                ones-block matmul, then normalize and ship zT (everything
                except that one matmul runs off the PE)."""
                fh, fj, fz, fr = st
                rp = ps_r.tile([128, 512], F32, tag="r", name="rbc")
                nc.tensor.matmul(rp[:], ones_blk[:, :], fr[:],
                                 start=True, stop=True)
                rcp = z_pool.tile([128, 512], F32, tag="rcp")
                nc.vector.reciprocal(rcp[:], rp[:])
                zt = z_pool.tile([128, 512], BF16, tag="ztile")
                nc.vector.tensor_mul(zt[:], fz[:], rcp[:])
                if fh == 3 and fj == 3:
                    for p in range(4):
                        nc.sync.dma_start(
                            agin_p[p][:, :], zt[:, p * 128:(p + 1) * 128])
                        nc.gpsimd.collective_compute(
                            "AllGather",
                            mybir.AluOpType.bypass,
                            replica_groups=GROUPS,
                            ins=[agin_p[p].ap().opt()],
                            outs=[agout_p[p].ap().opt()],
                        )
                else:
                    nc.sync.dma_start(agin[fh][fj][:, :], zt[:])
                    nc.gpsimd.collective_compute(
                        "AllGather",
                        mybir.AluOpType.bypass,
                        replica_groups=GROUPS,
                        ins=[agin[fh][fj].ap().opt()],
                        outs=[agout[fh][fj].ap().opt()],
                    )

            def emit_zg(qg):
                """Load the gathered zT tiles for one 512-wide q-chunk.
                zg[:, h, r, :] = zT of global head 4*r + h."""
                zg = zg_pool.tile([128, HPC, 4, 512], BF16, tag="zg",
                                  name=f"zg{qg}")
                nh = 3 if qg == 3 else HPC
                for h in range(nh):
                    nc.gpsimd.dma_start(
                        zg[:, h, :, :],
                        agout[h][qg].ap().rearrange("(r p) s -> p r s", p=128))
                if qg == 3:
                    for p in range(4):
                        nc.gpsimd.dma_start(
                            zg[:, 3, :, p * 128:(p + 1) * 128],
                            agout_p[p].ap()
                            .rearrange("(r p2) s -> p2 r s", p2=128))
                return zg

            def emit_outproj(qg, zg, finalize_after_qi=None):
                osb = out_pool.tile([128, 4, QSL], F32, tag="os",
                                    name=f"os{qg}")
                for qi in range(4):
                    psum = ps1.tile([128, QSL], F32, tag="ps1",
                                    name=f"po{qg}_{qi}")
                    for tt in range(N_HEADS):
                        r, h = tt // HPC, tt % HPC
                        nc.tensor.matmul(
                            psum[:],
                            zg[:, h, r, qi * 128:(qi + 1) * 128],
                            wo_sb[:, tt, :],
                            start=(tt == 0), stop=(tt == N_HEADS - 1))
                    nc.scalar.copy(osb[:, qi, :], psum[:])
                    if qi == finalize_after_qi and pend[0] is not None:
                        finalize_chunk(pend[0])  # last (h3, j3) AllGather
                        pend[0] = None
                    if qi % 2 == 1:
                        nc.sync.dma_start(
                            out_e[qg * 512 + (qi - 1) * 128:
                                  qg * 512 + (qi + 1) * 128, :]
                            .rearrange("(t p) d -> p t d", p=128),
                            osb[:, qi - 1:qi + 1, :])

            # ---- per head: interleave QT/KT chunk chains with attention
            # chunks of the same head so no PE instruction ever waits on
            # an ACT/DVE epilogue; head 3 additionally interleaves the
            # output projection for q-chunks whose AllGathers have landed.
            pend = [None]
            zgs = {}

            def emit_qkt_chain(h, sc, wq_sb, wk_sb, qt_sb, kt_sb):
                for proj in range(2):
                    w_t = wq_sb if proj == 0 else wk_sb
                    dst = qt_sb if proj == 0 else kt_sb
                    psum = ps1.tile([128, 512], F32, tag="ps1",
                                    name=f"pq{h}_{sc}_{proj}")
                    for dt in range(NDT):
                        nc.tensor.matmul(
                            psum[:],
                            w_t[:, dt, :],
                            xt_sb[dt][:, sc * 512:(sc + 1) * 512],
                            start=(dt == 0), stop=(dt == NDT - 1))
                    if proj == 0:
                        nc.scalar.activation(
                            dst[:, sc * 512:(sc + 1) * 512], psum[:],
                            mybir.ActivationFunctionType.Identity,
                            bias=bq_sb[:, h:h + 1], scale=SCALE)
                    else:
                        nc.scalar.activation(
                            dst[:, sc * 512:(sc + 1) * 512], psum[:],
                            mybir.ActivationFunctionType.Identity,
                            bias=bk_sb[:, h:h + 1], scale=1.0)

            def emit_attn_chunk(h, j, qt_sb, kt_sb):
                n_st = 4 * (j + 1)
                ztp = ps_zt.tile([128, 512], F32, tag="zt",
                                 name=f"zt{h}_{j}")
                racc = ps_r.tile([128, 512], F32, tag="r", name=f"ra{h}_{j}")
                for i in range(n_st):
                    v = i - 4 * j
                    # causal: diagonal tile v touches only columns >= 128*v
                    c0 = 128 * v if v > 0 else 0
                    stp = ps_st.tile([128, 512], F32, tag="st")
                    nc.tensor.matmul(
                        stp[:, c0:],
                        kt_sb[:, i * 128:(i + 1) * 128],
                        qt_sb[:, j * 512 + c0:(j + 1) * 512],
                        start=True, stop=True)
                    pt = pt_pool.tile([128, 512], BF16, tag="pt")
                    nc.scalar.activation(
                        pt[:, c0:], stp[:, c0:],
                        mybir.ActivationFunctionType.Exp)
                    if v >= 0:
                        nc.vector.tensor_mul(
                            pt[:, c0:], pt[:, c0:],
                            mk_sb[:, v * 512 + c0:(v + 1) * 512])
                    nc.tensor.matmul(
                        ztp[:, c0:], v_sb[h][:, i, :], pt[:, c0:],
                        start=(i == 0), stop=(i == n_st - 1))
                    if i == 0:
                        nc.vector.tensor_copy(racc[:], pt[:])
                    else:
                        nc.vector.tensor_add(
                            racc[:, c0:], racc[:, c0:], pt[:, c0:])
                    if i == 1 and pend[0] is not None:
                        finalize_chunk(pend[0])
                        pend[0] = None
                racc_sb = z_pool.tile([128, 512], BF16, tag="racc_sb")
                nc.scalar.copy(racc_sb[:], racc[:])
                pend[0] = (h, j, ztp, racc_sb)

            for h in range(HPC):
                if h == 1:
                    # W_O loads deferred past the startup DMA crunch
                    for g4 in range(4):
                        nc.gpsimd.dma_start(
                            wo_sb[:, g4 * 4:(g4 + 1) * 4, :],
                            wo_e[g4 * 512:(g4 + 1) * 512, :]
                            .rearrange("(t p) d -> p t d", p=128))
                if h == 0:
                    wq_sb, wk_sb = wqk0
                else:
                    wq_sb = qk_pool.tile([128, NDT, D_HEAD], BF16, tag="wq",
                                         name=f"wq{h}")
                    nc.gpsimd.dma_start(
                        wq_sb[:],
                        wq_e[h * D_MODEL:(h + 1) * D_MODEL, :]
                        .rearrange("(t p) k -> p t k", p=128))
                    wk_sb = qk_pool.tile([128, NDT, D_HEAD], BF16, tag="wk",
                                         name=f"wk{h}")
                    nc.gpsimd.dma_start(
                        wk_sb[:],
                        wk_e[h * D_MODEL:(h + 1) * D_MODEL, :]
                        .rearrange("(t p) k -> p t k", p=128))
                qt_sb = qk_pool.tile([128, SEQ], BF16, tag="qt", name=f"qt{h}")
                kt_sb = qk_pool.tile([128, SEQ], BF16, tag="kt", name=f"kt{h}")

                if h == 0:
                    # interleave the V chains into head 0's stream
                    emit_qkt_chain(h, 0, wq_sb, wk_sb, qt_sb, kt_sb)
                    emit_v_chains(0, 4)
                    emit_qkt_chain(h, 1, wq_sb, wk_sb, qt_sb, kt_sb)
                    emit_v_chains(4, 8)
                    emit_attn_chunk(h, 0, qt_sb, kt_sb)
                    emit_qkt_chain(h, 2, wq_sb, wk_sb, qt_sb, kt_sb)
                    emit_v_chains(8, 12)
                    emit_attn_chunk(h, 1, qt_sb, kt_sb)
                    emit_qkt_chain(h, 3, wq_sb, wk_sb, qt_sb, kt_sb)
                    emit_v_chains(12, 16)
                    emit_attn_chunk(h, 2, qt_sb, kt_sb)
                    emit_attn_chunk(h, 3, qt_sb, kt_sb)
                    continue
                emit_qkt_chain(h, 0, wq_sb, wk_sb, qt_sb, kt_sb)
                emit_qkt_chain(h, 1, wq_sb, wk_sb, qt_sb, kt_sb)
                emit_attn_chunk(h, 0, qt_sb, kt_sb)
                emit_qkt_chain(h, 2, wq_sb, wk_sb, qt_sb, kt_sb)
                emit_attn_chunk(h, 1, qt_sb, kt_sb)
                emit_qkt_chain(h, 3, wq_sb, wk_sb, qt_sb, kt_sb)
                emit_attn_chunk(h, 2, qt_sb, kt_sb)
                if h == 3:
                    # q-chunk 0's quarter-AG landed during chunk 1; its
                    # zg loads were queued behind that AG on gpsimd.
                    zgs[0] = emit_zg(0)
                    emit_outproj(0, zgs[0])
                emit_attn_chunk(h, 3, qt_sb, kt_sb)
                if h == 3:
                    zgs[1] = emit_zg(1)

            emit_outproj(1, zgs[1], finalize_after_qi=0)
            zgs[2] = emit_zg(2)
            zgs[3] = emit_zg(3)
            emit_outproj(2, zgs[2])
            emit_outproj(3, zgs[3])
    nc.finalize()
    return nc


def _build_masks():
    """mask_v[r, c] = 1 if key position (128*v + r) <= query position c."""
    m = np.zeros((128, 4 * 512), dtype=NPBF16)
    r = np.arange(128)[:, None]
    c = np.arange(512)[None, :]
    for v in range(4):
        m[:, v * 512:(v + 1) * 512] = (c >= 128 * v + r).astype(NPBF16)
    return m


_NC_CACHE = None


def kernel(normalized_resid_pre, W_Q, b_Q, W_K, b_K, W_V, b_V, W_O, b_O):
    global LAST_EXEC_NS, _NC_CACHE
    x = np.asarray(normalized_resid_pre, dtype=np.float32)
    W_Q = np.asarray(W_Q, np.float32); b_Q = np.asarray(b_Q, np.float32)
    W_K = np.asarray(W_K, np.float32); b_K = np.asarray(b_K, np.float32)
    W_V = np.asarray(W_V, np.float32); b_V = np.asarray(b_V, np.float32)
    W_O = np.asarray(W_O, np.float32); b_O = np.asarray(b_O, np.float32)

    mask_m = _build_masks()
    wo_flat = W_O.reshape(N_HEADS * D_HEAD, D_MODEL)
    xt = [np.ascontiguousarray(x[b].T).astype(NPBF16) for b in range(B)]

    in_maps = []
    for c in range(NCORES):
        beta, g = c // 4, c % 4
        hs = slice(HPC * g, HPC * g + HPC)
        wq_m = np.ascontiguousarray(
            W_Q[hs].reshape(HPC * D_MODEL, D_HEAD)).astype(NPBF16)
        wk_m = np.ascontiguousarray(
            W_K[hs].reshape(HPC * D_MODEL, D_HEAD)).astype(NPBF16)
        wv_m = np.ascontiguousarray(
            W_V[hs].transpose(1, 0, 2).reshape(D_MODEL, HPC * D_HEAD)).astype(NPBF16)
        wo_m = np.ascontiguousarray(
            wo_flat[:, QSL * g:QSL * (g + 1)]).astype(NPBF16)
        bq_m = np.ascontiguousarray((b_Q[hs] * SCALE).T).astype(np.float32)
        bk_m = np.ascontiguousarray(b_K[hs].T).astype(np.float32)
        in_maps.append({
            "xt": xt[beta], "wq": wq_m, "wk": wk_m, "wv": wv_m,
            "wo": wo_m, "bq": bq_m, "bk": bk_m, "mk": mask_m,
        })

    if _NC_CACHE is None:
        _NC_CACHE = build_nc()
    nc = _NC_CACHE

    trace = False
    if os.environ.get("BASS_KERNEL_TRACE") == "1":
        try:
            from antenv.axon_hooks import get_axon_ntff_profile_hook
            trace = get_axon_ntff_profile_hook() is not None
        except ImportError:
            trace = False

    res = run_bass_kernel_spmd(nc, in_maps, core_ids=list(range(NCORES)),
                               trace=trace)
    LAST_EXEC_NS = res.exec_time_ns

    # bias correction: softmax rows sum to 1 -> b_V contributes a constant
    # row through W_O; b_O is a plain add.
    corr = b_O + np.einsum("hk,hkd->d", b_V, W_O)

    out = np.empty((B, SEQ, D_MODEL), dtype=np.float32)
    for c in range(NCORES):
        beta, g = c // 4, c % 4
        out[beta, :, QSL * g:QSL * (g + 1)] = (
            res.results[c]["out"] + corr[QSL * g:QSL * (g + 1)])
    return out


# revision 44
# speedup vs baseline: 1.0488x; 1.0488x over previous
"""Distributed causal multi-head attention for TRN2, 8 NeuronCores.

Sharding: core c (0..7) handles batch c//4 and heads 4*(c%4)..4*(c%4)+3
(tensor-parallel over heads x data-parallel over batch).

Per-core pipeline (all matmuls bf16, fp32 PSUM accumulate):
  1. QKV projections from host-pretransposed xT:
       QT/KT[k,s] = (W.T x.T) with W tiles stationary;
       V[s,k] with xT tiles stationary (4 heads packed in the free dim).
  2. Attention per head, scores transposed: ST[s,q] = KT.T @ QT.
     exp on ACT, causal masking by precomputed 0/1 tiles, then
     z[q,k] and the softmax row-sum r[q] in ONE matmul per
     (q-tile, s-tile): rhs = [V | ones] (129 columns).
     Normalize z by 1/r (per-partition scalar), PE-transpose to zT and
     DMA to the AllGather buffer in [hk_local, q] layout.
  3. AllGather over the 4-core batch group: every core gets the full
     zT [16*128, 2048] for its batch.
  4. Output projection, d-sharded: each core's wo input holds only its
     512 W_O columns, so out[all q, d_slice] = z_flat @ W_O[:, slice].
     The graph is identical on all cores; per-core behavior comes only
     from input data (SPMD-safe).

Host: shards/casts/transposes inputs, adds bias corrections
(b_O + sum_h b_V[h] @ W_O[h] is a constant row because softmax rows sum
to 1; b_Q/b_K are folded into the QT/KT PSUM evacuation on device).
"""
import math
import os

import numpy as np
import ml_dtypes

import concourse.bacc as bacc
import concourse.mybir as mybir
from concourse import tile, masks
from concourse.bass_utils import run_bass_kernel_spmd

BF16 = mybir.dt.bfloat16
F32 = mybir.dt.float32
NPBF16 = ml_dtypes.bfloat16

B = 2
SEQ = 2048
D_MODEL = 2048
N_HEADS = 16
D_HEAD = 128
HPC = 4              # heads per core
NCORES = 8
GROUPS = [[0, 1, 2, 3], [4, 5, 6, 7]]
NDT = D_MODEL // 128   # 16 d-model tiles
NST = SEQ // 128       # 16 seq tiles
NQC = SEQ // 512       # 4 q-chunks
QSL = SEQ // 4         # 512 per-core q-slice for output projection
SCALE = 1.0 / math.sqrt(D_HEAD)

LAST_EXEC_NS = None


def build_nc():
    nc = bacc.Bacc(None, num_devices=NCORES, debug=False)

    xt_e = nc.declare_dram_parameter("xt", [D_MODEL, SEQ], BF16, isOutput=False)
    wq_e = nc.declare_dram_parameter("wq", [HPC * D_MODEL, D_HEAD], BF16, isOutput=False)
    wk_e = nc.declare_dram_parameter("wk", [HPC * D_MODEL, D_HEAD], BF16, isOutput=False)
    wv_e = nc.declare_dram_parameter("wv", [D_MODEL, HPC * D_HEAD], BF16, isOutput=False)
    wo_e = nc.declare_dram_parameter("wo", [N_HEADS * D_HEAD, QSL], BF16, isOutput=False)
    bq_e = nc.declare_dram_parameter("bq", [D_HEAD, HPC], F32, isOutput=False)
    bk_e = nc.declare_dram_parameter("bk", [D_HEAD, HPC], F32, isOutput=False)
    mk_e = nc.declare_dram_parameter("mk", [128, 4 * 512], BF16, isOutput=False)
    out_e = nc.declare_dram_parameter("out", [SEQ, QSL], F32, isOutput=True)

    # AllGather buffers: one per (local head, q-chunk). Quarters are
    # cheap on the CC engine and keep its queue from head-blocking the
    # final gathers that gate the output projection.
    # heads 0-2 gather per q-half (cheaper on the CC engine); head 3
    # per q-quarter so the output projection can chase its chunks
    agin = [[nc.dram_tensor(f"agin{h}_{hf}", [D_HEAD, SEQ // 2], BF16)
             for hf in range(2)] for h in range(3)]
    agout = [[nc.dram_tensor(f"agout{h}_{hf}", [4 * D_HEAD, SEQ // 2], BF16)
              for hf in range(2)] for h in range(3)]
    agin3 = [nc.dram_tensor(f"agin3_{j}", [D_HEAD, 512], BF16)
             for j in range(3)]
    agout3 = [nc.dram_tensor(f"agout3_{j}", [4 * D_HEAD, 512], BF16)
              for j in range(3)]
    # the last (h3, j3) gather is split into 4 x 128-column pieces so the
    # final output-projection chains pipeline with the collective
    agin_p = [nc.dram_tensor(f"aginp{p}", [D_HEAD, 128], BF16)
              for p in range(4)]
    agout_p = [nc.dram_tensor(f"agoutp{p}", [4 * D_HEAD, 128], BF16)
               for p in range(4)]

    with tile.TileContext(nc) as tc:
        with tc.tile_pool(name="persist", bufs=1) as pp, \
             tc.tile_pool(name="xtp", bufs=NDT) as xt_pool, \
             tc.tile_pool(name="qkp", bufs=2) as qk_pool, \
             tc.tile_pool(name="vp", bufs=HPC) as v_pool, \
             tc.tile_pool(name="wvp", bufs=1) as wv_pool, \
             tc.tile_pool(name="pt", bufs=5) as pt_pool, \
             tc.tile_pool(name="zz", bufs=3) as z_pool, \
             tc.tile_pool(name="wo", bufs=1) as wo_pool, \
             tc.tile_pool(name="zg", bufs=2) as zg_pool, \
             tc.tile_pool(name="os", bufs=1) as out_pool, \
             tc.tile_pool(name="ps1", bufs=2, space="PSUM") as ps1, \
             tc.tile_pool(name="ps_st", bufs=2, space="PSUM") as ps_st, \
             tc.tile_pool(name="ps_zt", bufs=2, space="PSUM") as ps_zt, \
             tc.tile_pool(name="ps_r", bufs=2, space="PSUM") as ps_r:
            ones_blk = pp.tile([128, 128], BF16, tag="ones_blk")
            nc.vector.memset(ones_blk[:], 1.0)
            bq_sb = pp.tile([128, HPC], F32, tag="bq")
            nc.sync.dma_start(bq_sb[:], bq_e[:, :])
            bk_sb = pp.tile([128, HPC], F32, tag="bk")
            nc.sync.dma_start(bk_sb[:], bk_e[:, :])
            mk_sb = pp.tile([128, 4 * 512], BF16, tag="mk")
            nc.sync.dma_start(mk_sb[:], mk_e[:, :])

            xt_sb = []
            for dt in range(NDT):
                t = xt_pool.tile([128, SEQ], BF16, tag="xt", name=f"xt{dt}")
                eng = nc.sync if dt % 2 == 0 else nc.gpsimd
                eng.dma_start(t[:], xt_e[dt * 128:(dt + 1) * 128, :])
                xt_sb.append(t)

            # head 0's projection weights go first so its QT/KT chains can
            # start as soon as xt lands.
            wqk0 = []
            for nm, e in (("wq0", wq_e), ("wk0", wk_e)):
                t = qk_pool.tile([128, NDT, D_HEAD], BF16,
                                 tag="wq" if nm == "wq0" else "wk", name=nm)
                nc.gpsimd.dma_start(
                    t[:], e[0:D_MODEL, :].rearrange("(t p) k -> p t k", p=128))
                wqk0.append(t)

            # ---- V for all 4 heads (packed rhs) --------------------------
            v_sb = [v_pool.tile([128, NST, D_HEAD], BF16, tag="v",
                                name=f"v{h}") for h in range(HPC)]
            wv_sb = wv_pool.tile([128, NDT, HPC * D_HEAD], BF16, tag="wv")
            for g4 in range(4):
                nc.gpsimd.dma_start(
                    wv_sb[:, g4 * 4:(g4 + 1) * 4, :],
                    wv_e[g4 * 512:(g4 + 1) * 512, :]
                    .rearrange("(t p) k -> p t k", p=128))
            wo_sb = wo_pool.tile([128, N_HEADS, QSL], BF16, tag="wo")

            def emit_v_chains(st0, st1):
                for st in range(st0, st1):
                    psum = ps1.tile([128, 512], F32, tag="ps1",
                                    name=f"pv{st}")
                    for dt in range(NDT):
                        nc.tensor.matmul(
                            psum[:],
                            xt_sb[dt][:, st * 128:(st + 1) * 128],
                            wv_sb[:, dt, :],
                            start=(dt == 0), stop=(dt == NDT - 1))
                    for h in range(HPC):
                        nc.scalar.copy(
                            v_sb[h][:, st, 0:D_HEAD],
                            psum[:, h * 128:(h + 1) * 128])

            # ---- helpers -------------------------------------------------
            def finalize_chunk(st):
                """Chunk epilogue: broadcast the softmax denominator with a
                ones-block matmul, then normalize and ship zT (everything
                except that one matmul runs off the PE)."""
                fh, fj, fz, fr = st
                rp = ps_r.tile([128, 512], F32, tag="r", name="rbc")
                nc.tensor.matmul(rp[:], ones_blk[:, :], fr[:],
                                 start=True, stop=True)
                rcp = z_pool.tile([128, 512], F32, tag="rcp")
                nc.vector.reciprocal(rcp[:], rp[:])
                zt = z_pool.tile([128, 512], BF16, tag="ztile")
                nc.vector.tensor_mul(zt[:], fz[:], rcp[:])
                if fh == 3 and fj == 3:
                    for p in range(4):
                        nc.sync.dma_start(
                            agin_p[p][:, :], zt[:, p * 128:(p + 1) * 128])
                        nc.gpsimd.collective_compute(
                            "AllGather",
                            mybir.AluOpType.bypass,
                            replica_groups=GROUPS,
                            ins=[agin_p[p].ap().opt()],
                            outs=[agout_p[p].ap().opt()],
                        )
                elif fh == 3:
                    nc.sync.dma_start(agin3[fj][:, :], zt[:])
                    nc.gpsimd.collective_compute(
                        "AllGather",
                        mybir.AluOpType.bypass,
                        replica_groups=GROUPS,
                        ins=[agin3[fj].ap().opt()],
                        outs=[agout3[fj].ap().opt()],
                    )
                else:
                    nc.sync.dma_start(
                        agin[fh][fj // 2][:, (fj % 2) * 512:(fj % 2 + 1) * 512],
                        zt[:])
                    if fj % 2 == 1:
                        nc.gpsimd.collective_compute(
                            "AllGather",
                            mybir.AluOpType.bypass,
                            replica_groups=GROUPS,
                            ins=[agin[fh][fj // 2].ap().opt()],
                            outs=[agout[fh][fj // 2].ap().opt()],
                        )

            def emit_zg(qg):
                """Load the gathered zT tiles for one 512-wide q-chunk.
                zg[:, h, r, :] = zT of global head 4*r + h."""
                zg = zg_pool.tile([128, HPC, 4, 512], BF16, tag="zg",
                                  name=f"zg{qg}")
                hf, qc = qg // 2, qg % 2
                for h in range(3):
                    nc.gpsimd.dma_start(
                        zg[:, h, :, :],
                        agout[h][hf][:, qc * 512:(qc + 1) * 512]
                        .rearrange("(r p) s -> p r s", p=128))
                if qg == 3:
                    for p in range(4):
                        nc.gpsimd.dma_start(
                            zg[:, 3, :, p * 128:(p + 1) * 128],
                            agout_p[p].ap()
                            .rearrange("(r p2) s -> p2 r s", p2=128))
                else:
                    nc.gpsimd.dma_start(
                        zg[:, 3, :, :],
                        agout3[qg].ap().rearrange("(r p) s -> p r s", p=128))
                return zg

            def emit_outproj(qg, zg, finalize_after_qi=None):
                osb = out_pool.tile([128, 4, QSL], F32, tag="os",
                                    name=f"os{qg}")
                for qi in range(4):
                    psum = ps1.tile([128, QSL], F32, tag="ps1",
                                    name=f"po{qg}_{qi}")
                    for tt in range(N_HEADS):
                        r, h = tt // HPC, tt % HPC
                        nc.tensor.matmul(
                            psum[:],
                            zg[:, h, r, qi * 128:(qi + 1) * 128],
                            wo_sb[:, tt, :],
                            start=(tt == 0), stop=(tt == N_HEADS - 1))
                    nc.scalar.copy(osb[:, qi, :], psum[:])
                    if qi == finalize_after_qi and pend[0] is not None:
                        finalize_chunk(pend[0])  # last (h3, j3) AllGather
                        pend[0] = None
                    if qi % 2 == 1:
                        nc.sync.dma_start(
                            out_e[qg * 512 + (qi - 1) * 128:
                                  qg * 512 + (qi + 1) * 128, :]
                            .rearrange("(t p) d -> p t d", p=128),
                            osb[:, qi - 1:qi + 1, :])

            # ---- per head: interleave QT/KT chunk chains with attention
            # chunks of the same head so no PE instruction ever waits on
            # an ACT/DVE epilogue; head 3 additionally interleaves the
            # output projection for q-chunks whose AllGathers have landed.
            pend = [None]
            zgs = {}

            def emit_qkt_chain(h, sc, wq_sb, wk_sb, qt_sb, kt_sb):
                for proj in range(2):
                    w_t = wq_sb if proj == 0 else wk_sb
                    dst = qt_sb if proj == 0 else kt_sb
                    psum = ps1.tile([128, 512], F32, tag="ps1",
                                    name=f"pq{h}_{sc}_{proj}")
                    for dt in range(NDT):
                        nc.tensor.matmul(
                            psum[:],
                            w_t[:, dt, :],
                            xt_sb[dt][:, sc * 512:(sc + 1) * 512],
                            start=(dt == 0), stop=(dt == NDT - 1))
                    if proj == 0:
                        nc.scalar.activation(
                            dst[:, sc * 512:(sc + 1) * 512], psum[:],
                            mybir.ActivationFunctionType.Identity,
                            bias=bq_sb[:, h:h + 1], scale=SCALE)
                    else:
                        nc.scalar.activation(
                            dst[:, sc * 512:(sc + 1) * 512], psum[:],
                            mybir.ActivationFunctionType.Identity,
                            bias=bk_sb[:, h:h + 1], scale=1.0)

            def emit_attn_chunk(h, j, qt_sb, kt_sb):
                n_st = 4 * (j + 1)
                ztp = ps_zt.tile([128, 512], F32, tag="zt",
                                 name=f"zt{h}_{j}")
                racc = ps_r.tile([128, 512], F32, tag="r", name=f"ra{h}_{j}")
                for i in range(n_st):
                    v = i - 4 * j
                    # causal: diagonal tile v touches only columns >= 128*v
                    c0 = 128 * v if v > 0 else 0
                    stp = ps_st.tile([128, 512], F32, tag="st")
                    nc.tensor.matmul(
                        stp[:, c0:],
                        kt_sb[:, i * 128:(i + 1) * 128],
                        qt_sb[:, j * 512 + c0:(j + 1) * 512],
                        start=True, stop=True)
                    pt = pt_pool.tile([128, 512], BF16, tag="pt")
                    nc.scalar.activation(
                        pt[:, c0:], stp[:, c0:],
                        mybir.ActivationFunctionType.Exp)
                    if v >= 0:
                        nc.vector.tensor_mul(
                            pt[:, c0:], pt[:, c0:],
                            mk_sb[:, v * 512 + c0:(v + 1) * 512])
                    nc.tensor.matmul(
                        ztp[:, c0:], v_sb[h][:, i, :], pt[:, c0:],
                        start=(i == 0), stop=(i == n_st - 1))
                    if i == 0:
                        nc.vector.tensor_copy(racc[:], pt[:])
                    else:
                        nc.vector.tensor_add(
                            racc[:, c0:], racc[:, c0:], pt[:, c0:])
                    if i == 1 and pend[0] is not None:
                        finalize_chunk(pend[0])
                        pend[0] = None
                racc_sb = z_pool.tile([128, 512], BF16, tag="racc_sb")
                nc.scalar.copy(racc_sb[:], racc[:])
                pend[0] = (h, j, ztp, racc_sb)

            for h in range(HPC):
                if h == 1:
                    # W_O loads deferred past the startup DMA crunch
                    for g4 in range(4):
                        nc.gpsimd.dma_start(
                            wo_sb[:, g4 * 4:(g4 + 1) * 4, :],
                            wo_e[g4 * 512:(g4 + 1) * 512, :]
                            .rearrange("(t p) d -> p t d", p=128))
                if h == 0:
                    wq_sb, wk_sb = wqk0
                else:
                    wq_sb = qk_pool.tile([128, NDT, D_HEAD], BF16, tag="wq",
                                         name=f"wq{h}")
                    nc.gpsimd.dma_start(
                        wq_sb[:],
                        wq_e[h * D_MODEL:(h + 1) * D_MODEL, :]
                        .rearrange("(t p) k -> p t k", p=128))
                    wk_sb = qk_pool.tile([128, NDT, D_HEAD], BF16, tag="wk",
                                         name=f"wk{h}")
                    nc.gpsimd.dma_start(
                        wk_sb[:],
                        wk_e[h * D_MODEL:(h + 1) * D_MODEL, :]
                        .rearrange("(t p) k -> p t k", p=128))
                qt_sb = qk_pool.tile([128, SEQ], BF16, tag="qt", name=f"qt{h}")
                kt_sb = qk_pool.tile([128, SEQ], BF16, tag="kt", name=f"kt{h}")

                if h == 0:
                    # interleave the V chains into head 0's stream
                    emit_qkt_chain(h, 0, wq_sb, wk_sb, qt_sb, kt_sb)
                    emit_v_chains(0, 4)
                    emit_qkt_chain(h, 1, wq_sb, wk_sb, qt_sb, kt_sb)
                    emit_v_chains(4, 8)
                    emit_attn_chunk(h, 0, qt_sb, kt_sb)
                    emit_qkt_chain(h, 2, wq_sb, wk_sb, qt_sb, kt_sb)
                    emit_v_chains(8, 12)
                    emit_attn_chunk(h, 1, qt_sb, kt_sb)
                    emit_qkt_chain(h, 3, wq_sb, wk_sb, qt_sb, kt_sb)
                    emit_v_chains(12, 16)
                    emit_attn_chunk(h, 2, qt_sb, kt_sb)
                    emit_attn_chunk(h, 3, qt_sb, kt_sb)
                    continue
                emit_qkt_chain(h, 0, wq_sb, wk_sb, qt_sb, kt_sb)
                emit_qkt_chain(h, 1, wq_sb, wk_sb, qt_sb, kt_sb)
                emit_attn_chunk(h, 0, qt_sb, kt_sb)
                emit_qkt_chain(h, 2, wq_sb, wk_sb, qt_sb, kt_sb)
                emit_attn_chunk(h, 1, qt_sb, kt_sb)
                emit_qkt_chain(h, 3, wq_sb, wk_sb, qt_sb, kt_sb)
                emit_attn_chunk(h, 2, qt_sb, kt_sb)
                if h == 3:
                    # q-chunk 0's quarter-AG landed during chunk 1; its
                    # zg loads were queued behind that AG on gpsimd.
                    zgs[0] = emit_zg(0)
                    emit_outproj(0, zgs[0])
                emit_attn_chunk(h, 3, qt_sb, kt_sb)
                if h == 3:
                    zgs[1] = emit_zg(1)

            emit_outproj(1, zgs[1], finalize_after_qi=0)
            zgs[2] = emit_zg(2)
            zgs[3] = emit_zg(3)
            emit_outproj(2, zgs[2])
            emit_outproj(3, zgs[3])
    nc.finalize()
    return nc


def _build_masks():
    """mask_v[r, c] = 1 if key position (128*v + r) <= query position c."""
    m = np.zeros((128, 4 * 512), dtype=NPBF16)
    r = np.arange(128)[:, None]
    c = np.arange(512)[None, :]
    for v in range(4):
        m[:, v * 512:(v + 1) * 512] = (c >= 128 * v + r).astype(NPBF16)
    return m


_NC_CACHE = None


def kernel(normalized_resid_pre, W_Q, b_Q, W_K, b_K, W_V, b_V, W_O, b_O):
    global LAST_EXEC_NS, _NC_CACHE
    x = np.asarray(normalized_resid_pre, dtype=np.float32)
    W_Q = np.asarray(W_Q, np.float32); b_Q = np.asarray(b_Q, np.float32)
    W_K = np.asarray(W_K, np.float32); b_K = np.asarray(b_K, np.float32)
    W_V = np.asarray(W_V, np.float32); b_V = np.asarray(b_V, np.float32)
    W_O = np.asarray(W_O, np.float32); b_O = np.asarray(b_O, np.float32)

    mask_m = _build_masks()
    wo_flat = W_O.reshape(N_HEADS * D_HEAD, D_MODEL)
    xt = [np.ascontiguousarray(x[b].T).astype(NPBF16) for b in range(B)]

    in_maps = []
    for c in range(NCORES):
        beta, g = c // 4, c % 4
        hs = slice(HPC * g, HPC * g + HPC)
        wq_m = np.ascontiguousarray(
            W_Q[hs].reshape(HPC * D_MODEL, D_HEAD)).astype(NPBF16)
        wk_m = np.ascontiguousarray(
            W_K[hs].reshape(HPC * D_MODEL, D_HEAD)).astype(NPBF16)
        wv_m = np.ascontiguousarray(
            W_V[hs].transpose(1, 0, 2).reshape(D_MODEL, HPC * D_HEAD)).astype(NPBF16)
        wo_m = np.ascontiguousarray(
            wo_flat[:, QSL * g:QSL * (g + 1)]).astype(NPBF16)
        bq_m = np.ascontiguousarray((b_Q[hs] * SCALE).T).astype(np.float32)
        bk_m = np.ascontiguousarray(b_K[hs].T).astype(np.float32)
        in_maps.append({
            "xt": xt[beta], "wq": wq_m, "wk": wk_m, "wv": wv_m,
            "wo": wo_m, "bq": bq_m, "bk": bk_m, "mk": mask_m,
        })

    if _NC_CACHE is None:
        _NC_CACHE = build_nc()
    nc = _NC_CACHE

    trace = False
    if os.environ.get("BASS_KERNEL_TRACE") == "1":
        try:
            from antenv.axon_hooks import get_axon_ntff_profile_hook
            trace = get_axon_ntff_profile_hook() is not None
        except ImportError:
            trace = False

    res = run_bass_kernel_spmd(nc, in_maps, core_ids=list(range(NCORES)),
                               trace=trace)
    LAST_EXEC_NS = res.exec_time_ns

    # bias correction: softmax rows sum to 1 -> b_V contributes a constant
    # row through W_O; b_O is a plain add.
    corr = b_O + np.einsum("hk,hkd->d", b_V, W_O)

    out = np.empty((B, SEQ, D_MODEL), dtype=np.float32)
    for c in range(NCORES):
        beta, g = c // 4, c % 4
        out[beta, :, QSL * g:QSL * (g + 1)] = (
            res.results[c]["out"] + corr[QSL * g:QSL * (g + 1)])
    return out
